# revision 1
# baseline (speedup 1.0000x reference)
"""Confusion-matrix (joint histogram) kernel for Trainium2.

Math: out[b, i, j] = #{pixels p in batch b : yp[b,p] == i and y[b,p] == j}
for i, j in [0, 21). Inputs yp, y are [8, 2048, 2048] int32, values in [0, 21).

Per NeuronCore (core c processes batch c):
  - DMA int32 pixel chunks into SBUF, one ScalarE copy converts to bf16,
  - one-hot masks as {0,1} planes in matmul-ready interleaved layout
    (planes[p, blk*126 + i*6 + g]) via tensor_scalar(is_equal), classes
    split across VectorE and GpSimd,
  - joint counts via TensorE: confusion = onehot(yp)^T @ onehot(y), 6
    pixel-columns per matmul ([128, 126] x [128, 126]) accumulated into one
    PSUM [126, 126] f32 tile (exact integer counts < 2^24),
  - host extracts + sums the 6 diagonal 21x21 blocks.
"""

import numpy as np

C = 21                  # classes
G = 6                   # pixel-column groups per matmul (G*C = 126 <= 128)
M = G * C               # 126
P = 128                 # partitions
FP = 504                # plane-chunk columns per tensor (divisible by 6)
N_GP = 0                # mask classes on GpSimd (rest on DVE)
SENTINEL = 64           # int32 value outside [0, 21)
MASK_DT = "bf16"

_CACHE = {}


def _build(
    n_free,
    work_cols=None,
    repeat=1,
    skip_mm=False,
    n_cls=C,
    n_gp=N_GP,
    mask_dt=MASK_DT,
):
    import concourse.bacc as bacc
    import concourse.mybir as mybir
    import concourse.tile as tile
    from contextlib import nullcontext

    if work_cols is None:
        work_cols = n_free

    nc = bacc.Bacc(
        "TRN2",
        target_bir_lowering=False,
        debug=False,
        enable_asserts=False,
        num_devices=8,
    )
    yp = nc.dram_tensor("yp", [P, n_free], mybir.dt.int32, kind="ExternalInput").ap()
    y = nc.dram_tensor("y", [P, n_free], mybir.dt.int32, kind="ExternalInput").ap()
    out = nc.dram_tensor("out", [M, M], mybir.dt.float32, kind="ExternalOutput").ap()

    n_main = (work_cols // FP) * FP
    tail_cols = work_cols - n_main                   # < FP
    tail_pad = -tail_cols % G
    tail_w = tail_cols + tail_pad
    total_mms = (n_main // G) + (tail_w // G)

    mdt = {"bf16": mybir.dt.bfloat16, "fp8": mybir.dt.float8e4}[mask_dt]
    bf16 = mybir.dt.bfloat16
    f32 = mybir.dt.float32
    i32 = mybir.dt.int32
    Copy = mybir.ActivationFunctionType.Copy
    n_dve = max(0, n_cls - n_gp)

    with tile.TileContext(nc) as tc:
        with (
            tc.tile_pool(name="psum", bufs=1, space="PSUM") as psum_pool,
            tc.tile_pool(name="cat", bufs=3) as cat_pool,
            tc.tile_pool(name="planes", bufs=2) as plane_pool,
            tc.tile_pool(name="singles", bufs=1) as singles,
        ):
            acc = psum_pool.tile([M, M], f32)
            mm = 0
            rep_ctx = tc.For_i(0, repeat, 1) if repeat > 1 else nullcontext()

            with rep_ctx:

                def do_plane_chunk(cat32, w):
                    """cat32: [128, 2*w] int32 = [yp vals | y vals], w % 6 == 0.

                    planes[p, blk*126 + i*6 + g] = (vals[p, blk*6+g] == i),
                    blk in [0, 2*w/6). A-side = blks [0, w/6), B-side = rest.
                    Each matmul reads a contiguous [128, 126] slice.
                    """
                    nonlocal mm
                    nblk = 2 * w // G
                    cat16 = cat_pool.tile([P, 2 * FP], bf16, tag="cat16")
                    c16 = cat16[:, : 2 * w]
                    nc.scalar.activation(c16[:], cat32[:], Copy)
                    planes = plane_pool.tile([P, C * 2 * FP], mdt, tag="planes")
                    pl3 = planes[:, : nblk * M].rearrange("p (b f) -> p b f", f=M)
                    cat3 = c16[:].rearrange("p (b f) -> p b f", f=G)
                    for i in range(n_dve):
                        nc.vector.tensor_scalar(
                            pl3[:, :, i * G : (i + 1) * G],
                            cat3[:],
                            float(i),
                            None,
                            mybir.AluOpType.is_equal,
                        )
                    for i in range(n_dve, n_cls):
                        nc.gpsimd.tensor_scalar(
                            pl3[:, :, i * G : (i + 1) * G],
                            cat3[:],
                            float(i),
                            None,
                            mybir.AluOpType.is_equal,
                        )
                    half = (w // G) * M
                    for t in (range(0) if skip_mm else range(w // G)):
                        nc.tensor.matmul(
                            acc[:, :],
                            planes[:, t * M : (t + 1) * M],
                            planes[:, half + t * M : half + (t + 1) * M],
                            start=(mm == 0),
                            stop=(mm == total_mms - 1),
                        )
                        mm += 1

                off = 0
                while off < n_main:
                    cat32 = cat_pool.tile([P, 2 * FP], i32, tag="cat32")
                    nc.sync.dma_start(cat32[:, :FP], yp[:, off : off + FP])
                    nc.sync.dma_start(cat32[:, FP:], y[:, off : off + FP])
                    do_plane_chunk(cat32, FP)
                    off += FP

                if tail_cols:
                    ct = cat_pool.tile([P, 2 * FP], i32, tag="cat32")
                    ctw = ct[:, : 2 * tail_w]
                    if tail_pad:
                        nc.vector.memset(ctw[:], SENTINEL)
                    nc.sync.dma_start(
                        ctw[:, :tail_cols], yp[:, n_main : n_main + tail_cols]
                    )
                    nc.sync.dma_start(
                        ctw[:, tail_w : tail_w + tail_cols],
                        y[:, n_main : n_main + tail_cols],
                    )
                    do_plane_chunk(ctw, tail_w)

            assert skip_mm or mm == total_mms
            res = singles.tile([M, M], f32)
            if skip_mm:
                nc.vector.memset(res[:], 0.0)
            else:
                nc.vector.tensor_copy(res[:], acc[:, :])
            nc.sync.dma_start(out, res[:])

    nc.compile()
    return nc


def _get(n_free):
    if n_free not in _CACHE:
        _CACHE[n_free] = _build(n_free)
    return _CACHE[n_free]


def kernel(yp, y, res, n_classes, _trace=False):
    from concourse import bass_utils

    yp = np.ascontiguousarray(np.asarray(yp))
    y = np.ascontiguousarray(np.asarray(y))
    B = yp.shape[0]
    n_free = yp[0].size // P
    nc = _get(n_free)
    in_maps = [
        {"yp": yp[b].reshape(P, n_free), "y": y[b].reshape(P, n_free)}
        for b in range(B)
    ]
    r = bass_utils.run_bass_kernel_spmd(
        nc, in_maps, core_ids=list(range(B)), trace=_trace
    )
    outs = []
    for b in range(B):
        Pm = r.results[b]["out"].astype(np.float64)
        Rb = np.zeros((C, C), np.float64)
        for g in range(G):
            Rb += Pm[g::G, g::G]
        outs.append(Rb)
    res_np = np.stack(outs).astype(np.float32)
    if _trace:
        kernel._last_results = r
    return res_np



# revision 4
# speedup vs baseline: 1.2781x; 1.2781x over previous
"""Confusion-matrix (joint histogram) kernel for Trainium2.

Math: out[b, i, j] = #{pixels p in batch b : yp[b,p] == i and y[b,p] == j}
for i, j in [0, 21). Inputs yp, y are [8, 2048, 2048] int32, values in [0, 21).

Per NeuronCore (core c processes batch c), mixed-basis encoding:
each class-slot column s of a 128-wide block holds f_s(v) where
  slots 0..NS-1   : sign-range masks  S_{s+1}(v) = sign(v - s - 0.5)  (ACT,
                    reads int32 directly, one pass per slot)
  slots NS..19    : one-hot masks     [v == s]                        (DVE
                    is_equal at 4x mode; optionally a few on GpSimd)
  slot 20         : constant 1.0  (memset once per plane buffer)
  cols 126..127   : padding so weight blocks are 128 wide (enables the
                    compiler's fast-weight-load path); contents garbage,
                    confined to out rows 126/127 which the host ignores.

TensorE accumulates X' = F C F^T over all pixel blocks (G=6 pixel-column
groups per 128-wide block, moving operand 126 cols), where C is the true
confusion matrix and F the slot-encoding matrix. Host decodes
C = F^-1 X F^-T exactly in float64 (all X entries are integers < 2^24,
sign masks are +-1 so PSUM fp32 accumulation is exact).
"""

import numpy as np

C = 21                  # classes
G = 6                   # pixel-column groups per block
M = G * C               # 126 used columns
BLK = 128               # padded block width (weights FWL wants 128)
P = 128                 # partitions
FP = 840                # pixel-chunk columns per tensor (divisible by 6)
NS = 4                  # sign-mask slots computed on ACT (slots 0..NS-1)
N_GP = 0                # trailing delta slots computed on GpSimd
ONES_SLOT = C - 1       # slot 20: constant ones (marginals)
SENTINEL = 64           # int32 pad value outside [0, 21)
N_FREE = 32768          # 2048*2048 / 128

_CACHE = {}


def _f_matrix(ns=NS):
    """F[s, v] = f_s(v): slot-encoding matrix, and its value at SENTINEL."""
    F = np.zeros((C, C), dtype=np.float64)
    v = np.arange(C)
    for s in range(ns):
        F[s] = np.where(v >= s + 1, 1.0, -1.0)
    for s in range(ns, C - 1):
        F[s, s] = 1.0
    F[C - 1] = 1.0
    u = np.zeros(C, dtype=np.float64)
    u[:ns] = 1.0          # sign(SENTINEL - s - 0.5) = +1
    u[C - 1] = 1.0        # ones
    assert abs(np.linalg.det(F)) > 0.5
    return F, u


def _build(n_free=N_FREE, fp=FP, ns=NS, n_gp=N_GP):
    import concourse.bacc as bacc
    import concourse.mybir as mybir
    import concourse.tile as tile

    nc = bacc.Bacc(
        "TRN2",
        target_bir_lowering=False,
        debug=False,
        enable_asserts=False,
        num_devices=8,
    )
    yp = nc.dram_tensor("yp", [P, n_free], mybir.dt.int32, kind="ExternalInput").ap()
    y = nc.dram_tensor("y", [P, n_free], mybir.dt.int32, kind="ExternalInput").ap()
    out = nc.dram_tensor("out", [M, M], mybir.dt.float32, kind="ExternalOutput").ap()

    n_chunks = n_free // fp
    n_main = n_chunks * fp
    tail_cols = n_free - n_main                  # < fp
    tail_pad = -tail_cols % G
    tail_w = tail_cols + tail_pad
    total_mms = (n_main // G) + (tail_w // G)
    nblk_max = 2 * fp // G                       # blocks in a full chunk (both halves)

    bf16 = mybir.dt.bfloat16
    f32 = mybir.dt.float32
    i32 = mybir.dt.int32
    Copy = mybir.ActivationFunctionType.Copy
    Sign = mybir.ActivationFunctionType.Sign
    is_equal = mybir.AluOpType.is_equal

    with tile.TileContext(nc) as tc:
        with (
            tc.tile_pool(name="psum", bufs=1, space="PSUM") as psum_pool,
            tc.tile_pool(name="cat", bufs=3) as cat_pool,
            tc.tile_pool(name="singles", bufs=1) as singles,
        ):
            acc = psum_pool.tile([P, M], f32)
            # Per-slot bias vectors for the ACT sign masks (bias must be a
            # [128, 1] SBUF AP for non-Copy activations).
            sbias = singles.tile([P, max(ns, 1)], f32)
            for s in range(ns):
                nc.vector.memset(sbias[:, s : s + 1], -(s + 0.5))
            # Two persistent plane buffers (manual double-buffer) so the
            # ones-columns survive across chunks after a single memset.
            planes_bufs = []
            for bi in range(2):
                pb = singles.tile(
                    [P, nblk_max * BLK], bf16, tag=f"planes{bi}", name=f"planes{bi}"
                )
                pv = pb.rearrange("p (b f) -> p b f", f=BLK)
                nc.vector.memset(
                    pv[:, :, ONES_SLOT * G : (ONES_SLOT + 1) * G], 1.0
                )
                planes_bufs.append(pb)

            mm = 0
            ci = 0

            def do_chunk(cat32, w):
                """cat32: [128, 2*w] int32 = [yp vals | y vals], w % 6 == 0."""
                nonlocal mm, ci
                nblk = 2 * w // G
                half = w // G                       # yp blocks per half
                pb = planes_bufs[ci % 2]
                ci += 1
                pl3 = pb.rearrange("p (b f) -> p b f", f=BLK)[:, :nblk]
                cat16 = cat_pool.tile([P, 2 * fp], bf16, tag="cat16")
                c16 = cat16[:, : 2 * w]
                nc.scalar.activation(c16[:], cat32[:], Copy)
                c16_3 = c16[:].rearrange("p (b f) -> p b f", f=G)
                c32_3 = cat32[:].rearrange("p (b f) -> p b f", f=G)
                for s in range(ns):
                    nc.scalar.activation(
                        pl3[:, :, s * G : (s + 1) * G],
                        c32_3[:],
                        Sign,
                        bias=sbias[:, s : s + 1],
                    )
                n_dve_end = C - 1 - n_gp
                for s in range(ns, n_dve_end):
                    nc.vector.tensor_scalar(
                        pl3[:, :, s * G : (s + 1) * G],
                        c16_3[:],
                        float(s),
                        None,
                        is_equal,
                    )
                for s in range(n_dve_end, C - 1):
                    nc.gpsimd.tensor_scalar(
                        pl3[:, :, s * G : (s + 1) * G],
                        c16_3[:],
                        float(s),
                        None,
                        is_equal,
                    )
                for t in range(half):
                    nc.tensor.matmul(
                        acc[:, :],
                        pb[:, t * BLK : (t + 1) * BLK],
                        pb[:, (half + t) * BLK : (half + t) * BLK + M],
                        start=(mm == 0),
                        stop=(mm == total_mms - 1),
                    )
                    mm += 1

            off = 0
            while off < n_main:
                cat32 = cat_pool.tile([P, 2 * fp], i32, tag="cat32")
                nc.sync.dma_start(cat32[:, :fp], yp[:, off : off + fp])
                nc.sync.dma_start(cat32[:, fp:], y[:, off : off + fp])
                do_chunk(cat32, fp)
                off += fp

            if tail_cols:
                ct = cat_pool.tile([P, 2 * fp], i32, tag="cat32")
                ctw = ct[:, : 2 * tail_w]
                if tail_pad:
                    nc.vector.memset(ctw[:], SENTINEL)
                nc.sync.dma_start(
                    ctw[:, :tail_cols], yp[:, n_main : n_main + tail_cols]
                )
                nc.sync.dma_start(
                    ctw[:, tail_w : tail_w + tail_cols],
                    y[:, n_main : n_main + tail_cols],
                )
                do_chunk(ctw, tail_w)

            assert mm == total_mms
            res = singles.tile([M, M], f32)
            nc.vector.tensor_copy(res[:], acc[:M, :])
            nc.sync.dma_start(out, res[:])

    nc.compile()
    return nc


def _get(n_free):
    if n_free not in _CACHE:
        _CACHE[n_free] = _build(n_free)
    return _CACHE[n_free]


def kernel(yp, y, res, n_classes, _trace=False):
    from concourse import bass_utils

    yp = np.ascontiguousarray(np.asarray(yp))
    y = np.ascontiguousarray(np.asarray(y))
    B = yp.shape[0]
    n_free = yp[0].size // P
    nc = _get(n_free)
    in_maps = [
        {"yp": yp[b].reshape(P, n_free), "y": y[b].reshape(P, n_free)}
        for b in range(B)
    ]
    r = bass_utils.run_bass_kernel_spmd(
        nc, in_maps, core_ids=list(range(B)), trace=_trace
    )

    F, u = _f_matrix()
    tail_cols = n_free % FP
    npad = ((-tail_cols % G) * P) if tail_cols else 0
    pad_contrib = npad * np.outer(u, u)

    outs = []
    for b in range(B):
        Pm = r.results[b]["out"].astype(np.float64)
        X = np.zeros((C, C), np.float64)
        for g in range(G):
            X += Pm[g::G, g::G]
        X -= pad_contrib
        Cnt = np.linalg.solve(F, np.linalg.solve(F, X.T).T)
        outs.append(np.round(Cnt))
    res_np = np.stack(outs).astype(np.float32)
    if _trace:
        kernel._last_results = r
    return res_np
